# revision 7
# baseline (speedup 1.0000x reference)
"""Bass/Trainium2 kernel for nn_DiagonalTraining (per-anti-diagonal Linear).

Math: for each anti-diagonal i of x[B,S,S] (entries x[b,r,i-r], r<=i),
apply Linear_i (weights W[i,:i+1,:i+1], bias b[i,:i+1]) to the gathered
vector and scatter back reversed. Equivalent to:
    D[b,i,j] = x[b,j,i-j] (j<=i else 0)
    out[b,i,k] = sum_j W[i,k,j] * D[b,i,j] + b[i,k]
    new_x[b,r,c] = out[b,r+c,c] if r+c < S else x[b,r,c]

Device does the einsum (memory-bound: streams the valid triangle of W);
gather/scatter/bias are tiny O(S^2) host ops.

Sharding: interleaved over diagonals — core c owns i = c, c+8, ..., c+504
(slot m holds diagonal 8m+c, padded to length L=8(m+1)). All cores run one
identical SPMD program with near-identical work; padding rows/cols of W and
D are zero by construction so results are exact.

Device layout: W is host-packed into the exact SBUF image each matmul
wants ([j-partition, (chunk, k)] per slot), so every W load is one
dma_start with multi-KB contiguous per-partition descriptors. Four
consecutive slots share one PSUM bank via tile_position col-packing
(output partitions 32t..32t+8), giving PE col-group concurrency and a
128-partition PSUM->DRAM output DMA per group.
"""

import sys

sys.path.insert(0, "/opt/trn_rl_repo")

import numpy as np

B = 8
S = 512
NCORES = 8
M = 64  # diagonal slots per core
LBAR = [8 * (m + 1) for m in range(M)]  # padded diagonal length per slot
NQ = [1 if m < 16 else (m // 16 + 1) for m in range(M)]  # j-chunks per slot
# wimg column offsets: slot m's image is [128, NQ[m]*LBAR[m]]
WCUM = np.cumsum([0] + [NQ[m] * LBAR[m] for m in range(M)]).tolist()
WTOT = WCUM[M]  # 51840
SMALL_TOT = WCUM[16]  # 1088 (slots 0..15, single chunk, resident)
G = 16  # groups of 4 slots sharing a PSUM bank
LG = [32 * (g + 1) for g in range(G)]  # group output width
OCUM = np.cumsum([0] + LG).tolist()
OTOT = OCUM[G]  # 4352

# "fp32" (exact), "bf16" (half traffic), "fp32r" (full-rate fp32 PE mode)
MODE = "bf16"

_compiled = {}


def _np_dtype(mode):
    if mode == "bf16":
        import ml_dtypes

        return np.dtype(ml_dtypes.bfloat16)
    return np.dtype(np.float32)


def _bir_dtype(mode):
    import concourse.mybir as mybir

    return {
        "fp32": mybir.dt.float32,
        "fp32r": mybir.dt.float32r,
        "bf16": mybir.dt.bfloat16,
    }[mode]


def build_program(mode=MODE):
    """Build the SPMD Bass program (same instructions on all 8 cores)."""
    import concourse.mybir as mybir
    import concourse.tile as tile
    from concourse import bacc

    wdt = _bir_dtype(mode)
    f32 = mybir.dt.float32

    nc = bacc.Bacc("TRN2")
    wimg = nc.dram_tensor("wimg", [128, WTOT], wdt, kind="ExternalInput")
    dt_in = nc.dram_tensor("dt", [128, 4 * M * B], wdt, kind="ExternalInput")
    out = nc.dram_tensor("out", [128, OTOT], f32, kind="ExternalOutput")

    with tile.TileContext(nc) as tc:
        with (
            tc.tile_pool(name="dpool", bufs=1) as dpool,
            tc.tile_pool(name="wspool", bufs=1) as wspool,
            tc.tile_pool(name="wpool", bufs=10) as wpool,
            tc.tile_pool(name="opool", bufs=4) as opool,
            tc.tile_pool(name="psum", bufs=8, space="PSUM") as psum_pool,
        ):
            # Resident D^T image: [j-partition, (q, m, b)] — split across the
            # two HWDGE rings so the first matmuls aren't gated on one DMA.
            dtall = dpool.tile([128, 4 * M * B], wdt)
            half = 2 * M * B
            nc.sync.dma_start(dtall[:, 0:half], dt_in[:, 0:half])
            nc.scalar.dma_start(dtall[:, half:], dt_in[:, half:])
            # Resident packed W for small slots 0..15.
            wsmall = wspool.tile([128, SMALL_TOT], wdt)
            nc.sync.dma_start(wsmall[:], wimg[:, 0:SMALL_TOT])

            dma_engines = [nc.sync, nc.scalar]
            n_dma = 0

            # Largest groups first: dense PE work early (HAM warm-up),
            # small cheap groups last (short pipeline tail).
            for g in range(G - 1, -1, -1):
                ps = psum_pool.tile([128, 512], f32, tag="ps")
                for t in range(4):
                    m = 4 * g + t
                    L = LBAR[m]
                    nq = NQ[m]
                    if m < 16:
                        wt_ap = wsmall[0:128, WCUM[m] : WCUM[m] + L]
                    else:
                        wtile = wpool.tile([128, 2048], wdt, tag="w")
                        eng = dma_engines[n_dma % 2]
                        n_dma += 1
                        eng.dma_start(
                            wtile[0:128, 0 : nq * L], wimg[:, WCUM[m] : WCUM[m + 1]]
                        )
                    for q in range(nq):
                        rhs = (
                            wt_ap
                            if m < 16
                            else wtile[0:128, q * L : (q + 1) * L]
                        )
                        nc.tensor.matmul(
                            ps[32 * t : 32 * t + B, 0:L],
                            lhsT=dtall[0:128, q * M * B + m * B : q * M * B + (m + 1) * B],
                            rhs=rhs,
                            start=(q == 0),
                            stop=(q == nq - 1),
                            tile_position=(0, 32 * t),
                        )
                ot = opool.tile([128, 512], f32, tag="ostage")
                nc.vector.tensor_copy(ot[0:128, 0 : LG[g]], ps[0:128, 0 : LG[g]])
                nc.gpsimd.dma_start(
                    out[:, OCUM[g] : OCUM[g + 1]], ot[0:128, 0 : LG[g]]
                )

    nc.compile()
    return nc


def _get_program(mode=MODE):
    if mode not in _compiled:
        _compiled[mode] = build_program(mode)
    return _compiled[mode]


def _prep_inputs(x, W, mode=MODE):
    """Host-side shard prep: gather diagonals of x, pack W SBUF images."""
    ndt = _np_dtype(mode)
    i_idx = np.arange(S)[:, None]
    r_idx = np.arange(S)[None, :]
    cols = (i_idx - r_idx) % S
    valid = (r_idx <= i_idx)[None]
    D = np.where(valid, x[:, r_idx, cols], np.float32(0.0))  # [B, S(i), S(j)]

    in_maps = []
    for c in range(NCORES):
        Wc = W[c::8]  # [M, S(k), S(j)]
        WIMG = np.empty((128, WTOT), dtype=ndt)
        for m in range(M):
            L, nq = LBAR[m], NQ[m]
            # img[j, (q, k)] = Wc[m, k, 128q + j]
            blk = Wc[m, 0:L, 0 : 128 * nq].astype(ndt, copy=False)  # [k=L, j]
            img = blk.T.reshape(nq, 128, L).transpose(1, 0, 2).reshape(128, nq * L)
            WIMG[:, WCUM[m] : WCUM[m + 1]] = img
        # DT[j, q, m, b] = D[b, 8m+c, 128q+j]
        DT = np.ascontiguousarray(
            D[:, c::8, :].transpose(2, 1, 0).reshape(4, 128, M, B).transpose(1, 0, 2, 3)
        ).astype(ndt, copy=False)
        in_maps.append({"wimg": WIMG, "dt": DT.reshape(128, 4 * M * B)})
    return in_maps


def _postprocess(x, bvec, results):
    """Assemble per-core outputs, add bias, scatter back."""
    out_full = np.empty((B, S, S), dtype=np.float32)
    for c in range(NCORES):
        o = results[c]["out"]  # [128, OTOT]
        for g in range(G):
            blk = o[:, OCUM[g] : OCUM[g + 1]].reshape(4, 32, LG[g])[:, 0:B]
            for t in range(4):
                m = 4 * g + t
                out_full[:, 8 * m + c, 0 : LBAR[m]] = blk[t, :, 0 : LBAR[m]]
    out_full += bvec[None]
    rr = np.arange(S)[:, None]
    cc = np.arange(S)[None, :]
    diag = rr + cc
    new_x = np.where(
        (diag < S)[None], out_full[:, np.minimum(diag, S - 1), cc], x
    ).astype(np.float32)
    return new_x


def kernel_run(x, W, b, mode=MODE, trace=False):
    from concourse.bass_utils import run_bass_kernel_spmd

    nc = _get_program(mode)
    in_maps = _prep_inputs(x, W, mode)
    res = run_bass_kernel_spmd(nc, in_maps, list(range(NCORES)), trace=trace)
    return _postprocess(x, b, res.results), res


def kernel(x, W, b):
    out, _ = kernel_run(np.asarray(x), np.asarray(W), np.asarray(b))
    return out


# revision 10
# speedup vs baseline: 1.0003x; 1.0003x over previous
"""Bass/Trainium2 kernel for nn_DiagonalTraining (per-anti-diagonal Linear).

Math: for each anti-diagonal i of x[B,S,S] (entries x[b,r,i-r], r<=i),
apply Linear_i (weights W[i,:i+1,:i+1], bias b[i,:i+1]) to the gathered
vector and scatter back reversed. Equivalent to:
    D[b,i,j] = x[b,j,i-j] (j<=i else 0)
    out[b,i,k] = sum_j W[i,k,j] * D[b,i,j] + b[i,k]
    new_x[b,r,c] = out[b,r+c,c] if r+c < S else x[b,r,c]

Device does the einsum (memory-bound: streams the valid triangle of W);
gather/scatter/bias are tiny O(S^2) host ops.

Sharding: interleaved over diagonals — core c owns i = c, c+8, ..., c+504
(slot m holds diagonal 8m+c, padded to length L=8(m+1)). All cores run one
identical SPMD program with near-identical work; padding rows/cols of W and
D are zero by construction so results are exact.

Device layout: W is host-packed into the exact SBUF image each matmul
wants ([j-partition, (chunk, k)] per slot), so every W load is one
dma_start with multi-KB contiguous per-partition descriptors. Four
consecutive slots share one PSUM bank via tile_position col-packing
(output partitions 32t..32t+8), giving PE col-group concurrency and a
128-partition PSUM->DRAM output DMA per group.
"""

import sys

sys.path.insert(0, "/opt/trn_rl_repo")

import numpy as np

B = 8
S = 512
NCORES = 8
M = 64  # diagonal slots per core
LBAR = [8 * (m + 1) for m in range(M)]  # padded diagonal length per slot
NQ = [1 if m < 16 else (m // 16 + 1) for m in range(M)]  # j-chunks per slot
# wimg column offsets: slot m's image is [128, NQ[m]*LBAR[m]]
WCUM = np.cumsum([0] + [NQ[m] * LBAR[m] for m in range(M)]).tolist()
WTOT = WCUM[M]  # 51840
SMALL_TOT = WCUM[16]  # 1088 (slots 0..15, single chunk, resident)
G = 16  # groups of 4 slots sharing a PSUM bank
LG = [32 * (g + 1) for g in range(G)]  # group output width
OCUM = np.cumsum([0] + LG).tolist()
OTOT = OCUM[G]  # 4352

# "fp32" (exact), "bf16" (half traffic), "fp32r" (full-rate fp32 PE mode)
MODE = "bf16"

_compiled = {}


def _np_dtype(mode):
    if mode == "bf16":
        import ml_dtypes

        return np.dtype(ml_dtypes.bfloat16)
    return np.dtype(np.float32)


def _bir_dtype(mode):
    import concourse.mybir as mybir

    return {
        "fp32": mybir.dt.float32,
        "fp32r": mybir.dt.float32r,
        "bf16": mybir.dt.bfloat16,
    }[mode]


def build_program(mode=MODE):
    """Build the SPMD Bass program (same instructions on all 8 cores)."""
    import concourse.mybir as mybir
    import concourse.tile as tile
    from concourse import bacc

    wdt = _bir_dtype(mode)
    f32 = mybir.dt.float32

    nc = bacc.Bacc("TRN2")
    wimg = nc.dram_tensor("wimg", [128, WTOT], wdt, kind="ExternalInput")
    dt_in = nc.dram_tensor("dt", [128, 4 * M * B], wdt, kind="ExternalInput")
    out = nc.dram_tensor("out", [128, OTOT], f32, kind="ExternalOutput")

    with tile.TileContext(nc) as tc:
        with (
            tc.tile_pool(name="dpool", bufs=1) as dpool,
            tc.tile_pool(name="wspool", bufs=1) as wspool,
            tc.tile_pool(name="wpool", bufs=14) as wpool,
            tc.tile_pool(name="opool", bufs=4) as opool,
            tc.tile_pool(name="psum", bufs=8, space="PSUM") as psum_pool,
        ):
            # Resident D^T image: [j-partition, (q, m, b)] — split across the
            # two HWDGE rings so the first matmuls aren't gated on one DMA.
            dtall = dpool.tile([128, 4 * M * B], wdt)
            half = 2 * M * B
            nc.sync.dma_start(dtall[:, 0:half], dt_in[:, 0:half])
            # Resident packed W for small slots 0..15.
            wsmall = wspool.tile([128, SMALL_TOT], wdt)
            nc.scalar.dma_start(wsmall[:], wimg[:, 0:SMALL_TOT])
            nc.scalar.dma_start(dtall[:, half:], dt_in[:, half:])

            dma_engines = [nc.sync, nc.scalar]
            n_dma = 0

            # Small groups 3,2 first (their W is resident — instant PE work
            # while the stream ramps), then largest-first, tiny groups last
            # (short pipeline tail).
            for g in [3, 2] + list(range(G - 1, 3, -1)) + [1, 0]:
                ps = psum_pool.tile([128, 512], f32, tag="ps")
                for t in range(4):
                    m = 4 * g + t
                    L = LBAR[m]
                    nq = NQ[m]
                    if m < 16:
                        wt_ap = wsmall[0:128, WCUM[m] : WCUM[m] + L]
                    else:
                        wtile = wpool.tile([128, 2048], wdt, tag="w")
                        eng = dma_engines[n_dma % 2]
                        n_dma += 1
                        eng.dma_start(
                            wtile[0:128, 0 : nq * L], wimg[:, WCUM[m] : WCUM[m + 1]]
                        )
                    for q in range(nq):
                        rhs = (
                            wt_ap
                            if m < 16
                            else wtile[0:128, q * L : (q + 1) * L]
                        )
                        nc.tensor.matmul(
                            ps[32 * t : 32 * t + B, 0:L],
                            lhsT=dtall[0:128, q * M * B + m * B : q * M * B + (m + 1) * B],
                            rhs=rhs,
                            start=(q == 0),
                            stop=(q == nq - 1),
                            tile_position=(0, 32 * t),
                        )
                ot = opool.tile([128, 512], f32, tag="ostage")
                nc.vector.tensor_copy(ot[0:128, 0 : LG[g]], ps[0:128, 0 : LG[g]])
                nc.gpsimd.dma_start(
                    out[:, OCUM[g] : OCUM[g + 1]], ot[0:128, 0 : LG[g]]
                )

    nc.compile()
    return nc


def _get_program(mode=MODE):
    if mode not in _compiled:
        _compiled[mode] = build_program(mode)
    return _compiled[mode]


def _prep_inputs(x, W, mode=MODE):
    """Host-side shard prep: gather diagonals of x, pack W SBUF images."""
    ndt = _np_dtype(mode)
    i_idx = np.arange(S)[:, None]
    r_idx = np.arange(S)[None, :]
    cols = (i_idx - r_idx) % S
    valid = (r_idx <= i_idx)[None]
    D = np.where(valid, x[:, r_idx, cols], np.float32(0.0))  # [B, S(i), S(j)]

    in_maps = []
    for c in range(NCORES):
        Wc = W[c::8]  # [M, S(k), S(j)]
        WIMG = np.empty((128, WTOT), dtype=ndt)
        for m in range(M):
            L, nq = LBAR[m], NQ[m]
            # img[j, (q, k)] = Wc[m, k, 128q + j]
            blk = Wc[m, 0:L, 0 : 128 * nq].astype(ndt, copy=False)  # [k=L, j]
            img = blk.T.reshape(nq, 128, L).transpose(1, 0, 2).reshape(128, nq * L)
            WIMG[:, WCUM[m] : WCUM[m + 1]] = img
        # DT[j, q, m, b] = D[b, 8m+c, 128q+j]
        DT = np.ascontiguousarray(
            D[:, c::8, :].transpose(2, 1, 0).reshape(4, 128, M, B).transpose(1, 0, 2, 3)
        ).astype(ndt, copy=False)
        in_maps.append({"wimg": WIMG, "dt": DT.reshape(128, 4 * M * B)})
    return in_maps


def _postprocess(x, bvec, results):
    """Assemble per-core outputs, add bias, scatter back."""
    out_full = np.empty((B, S, S), dtype=np.float32)
    for c in range(NCORES):
        o = results[c]["out"]  # [128, OTOT]
        for g in range(G):
            blk = o[:, OCUM[g] : OCUM[g + 1]].reshape(4, 32, LG[g])[:, 0:B]
            for t in range(4):
                m = 4 * g + t
                out_full[:, 8 * m + c, 0 : LBAR[m]] = blk[t, :, 0 : LBAR[m]]
    out_full += bvec[None]
    rr = np.arange(S)[:, None]
    cc = np.arange(S)[None, :]
    diag = rr + cc
    new_x = np.where(
        (diag < S)[None], out_full[:, np.minimum(diag, S - 1), cc], x
    ).astype(np.float32)
    return new_x


def kernel_run(x, W, b, mode=MODE, trace=False):
    from concourse.bass_utils import run_bass_kernel_spmd

    nc = _get_program(mode)
    in_maps = _prep_inputs(x, W, mode)
    res = run_bass_kernel_spmd(nc, in_maps, list(range(NCORES)), trace=trace)
    return _postprocess(x, b, res.results), res


def kernel(x, W, b):
    out, _ = kernel_run(np.asarray(x), np.asarray(W), np.asarray(b))
    return out


# revision 12
# speedup vs baseline: 1.1614x; 1.1611x over previous
"""Bass/Trainium2 kernel for nn_DiagonalTraining (per-anti-diagonal Linear).

Math: for each anti-diagonal i of x[B,S,S] (entries x[b,r,i-r], r<=i),
apply Linear_i (weights W[i,:i+1,:i+1], bias b[i,:i+1]) to the gathered
vector and scatter back reversed. Equivalent to:
    D[b,i,j] = x[b,j,i-j] (j<=i else 0)
    out[b,i,k] = sum_j W[i,k,j] * D[b,i,j] + b[i,k]
    new_x[b,r,c] = out[b,r+c,c] if r+c < S else x[b,r,c]

Device does the einsum (memory-bound: streams the valid triangle of W);
gather/scatter/bias are tiny O(S^2) host ops.

Sharding: interleaved over diagonals — core c owns i = c, c+8, ..., c+504
(slot m holds diagonal 8m+c, padded to length L=8(m+1)). All cores run one
identical SPMD program with near-identical work; padding rows/cols of W and
D are zero by construction so results are exact.

Device layout: W is host-packed into the exact SBUF image each matmul
wants ([j-partition, (chunk, k)] per slot), so every W load is one
dma_start with multi-KB contiguous per-partition descriptors. Four
consecutive slots share one PSUM bank via tile_position col-packing
(output partitions 32t..32t+8), giving PE col-group concurrency and a
128-partition PSUM->DRAM output DMA per group.
"""

import sys

sys.path.insert(0, "/opt/trn_rl_repo")

import numpy as np

B = 8
S = 512
NCORES = 8
M = 64  # diagonal slots per core
LBAR = [8 * (m + 1) for m in range(M)]  # padded diagonal length per slot
NQ = [1 if m < 16 else (m // 16 + 1) for m in range(M)]  # j-chunks per slot
# wimg column offsets: slot m's image is [128, NQ[m]*LBAR[m]]
WCUM = np.cumsum([0] + [NQ[m] * LBAR[m] for m in range(M)]).tolist()
WTOT = WCUM[M]  # 51840
SMALL_TOT = WCUM[16]  # 1088 (slots 0..15, single chunk, resident)
G = 16  # groups of 4 slots sharing a PSUM bank
LG = [32 * (g + 1) for g in range(G)]  # group output width
OCUM = np.cumsum([0] + LG).tolist()
OTOT = OCUM[G]  # 4352

# "fp32" (exact), "bf16" (half traffic), "fp32r" (full-rate fp32 PE mode)
MODE = "bf16"

_compiled = {}


def _np_dtype(mode):
    if mode == "bf16":
        import ml_dtypes

        return np.dtype(ml_dtypes.bfloat16)
    return np.dtype(np.float32)


def _bir_dtype(mode):
    import concourse.mybir as mybir

    return {
        "fp32": mybir.dt.float32,
        "fp32r": mybir.dt.float32r,
        "bf16": mybir.dt.bfloat16,
    }[mode]


def build_program(mode=MODE):
    """Build the SPMD Bass program (same instructions on all 8 cores)."""
    import concourse.mybir as mybir
    import concourse.tile as tile
    from concourse import bacc

    wdt = _bir_dtype(mode)
    f32 = mybir.dt.float32

    nc = bacc.Bacc("TRN2")
    wimg = nc.dram_tensor("wimg", [128, WTOT], wdt, kind="ExternalInput")
    dt_in = nc.dram_tensor("dt", [128, 4 * M * B], wdt, kind="ExternalInput")
    out = nc.dram_tensor("out", [128, OTOT], f32, kind="ExternalOutput")

    with tile.TileContext(nc) as tc:
        with (
            tc.tile_pool(name="dpool", bufs=1) as dpool,
            tc.tile_pool(name="wspool", bufs=1) as wspool,
            tc.tile_pool(name="wpool", bufs=4) as wpool,
            tc.tile_pool(name="opool", bufs=4) as opool,
            tc.tile_pool(name="psum", bufs=8, space="PSUM") as psum_pool,
        ):
            # Resident D^T image: [j-partition, (q, m, b)] — split across the
            # two HWDGE rings so the first matmuls aren't gated on one DMA.
            dtall = dpool.tile([128, 4 * M * B], wdt)
            half = 2 * M * B
            nc.sync.dma_start(dtall[:, 0:half], dt_in[:, 0:half])
            # Resident packed W for small slots 0..15.
            wsmall = wspool.tile([128, SMALL_TOT], wdt)
            nc.scalar.dma_start(wsmall[:], wimg[:, 0:SMALL_TOT])
            nc.scalar.dma_start(dtall[:, half:], dt_in[:, half:])

            dma_engines = [nc.sync, nc.scalar]
            n_dma = 0

            # Small groups 3,2 first (their W is resident — instant PE work
            # while the stream ramps), then largest-first, tiny groups last
            # (short pipeline tail).
            for g in [3, 2] + list(range(G - 1, 3, -1)) + [1, 0]:
                ps = psum_pool.tile([128, 512], f32, tag="ps")
                if g >= 4:
                    # One DMA per group: the 4 members' images are adjacent
                    # in wimg, so this is a single 0.15-2MB transfer with
                    # multi-KB per-partition descriptors.
                    gw = WCUM[4 * g + 4] - WCUM[4 * g]
                    wtile = wpool.tile([128, 8000], wdt, tag="w")
                    eng = dma_engines[n_dma % 2]
                    n_dma += 1
                    eng.dma_start(
                        wtile[0:128, 0:gw], wimg[:, WCUM[4 * g] : WCUM[4 * g + 4]]
                    )
                for t in range(4):
                    m = 4 * g + t
                    L = LBAR[m]
                    nq = NQ[m]
                    if m < 16:
                        wt_ap = wsmall[0:128, WCUM[m] : WCUM[m] + L]
                        woff = 0
                    else:
                        woff = WCUM[m] - WCUM[4 * g]
                    for q in range(nq):
                        rhs = (
                            wt_ap
                            if m < 16
                            else wtile[0:128, woff + q * L : woff + (q + 1) * L]
                        )
                        nc.tensor.matmul(
                            ps[32 * t : 32 * t + B, 0:L],
                            lhsT=dtall[0:128, q * M * B + m * B : q * M * B + (m + 1) * B],
                            rhs=rhs,
                            start=(q == 0),
                            stop=(q == nq - 1),
                            tile_position=(0, 32 * t),
                        )
                ot = opool.tile([128, 512], f32, tag="ostage")
                nc.vector.tensor_copy(ot[0:128, 0 : LG[g]], ps[0:128, 0 : LG[g]])
                nc.gpsimd.dma_start(
                    out[:, OCUM[g] : OCUM[g + 1]], ot[0:128, 0 : LG[g]]
                )

    nc.compile()
    return nc


def _get_program(mode=MODE):
    if mode not in _compiled:
        _compiled[mode] = build_program(mode)
    return _compiled[mode]


def _prep_inputs(x, W, mode=MODE):
    """Host-side shard prep: gather diagonals of x, pack W SBUF images."""
    ndt = _np_dtype(mode)
    i_idx = np.arange(S)[:, None]
    r_idx = np.arange(S)[None, :]
    cols = (i_idx - r_idx) % S
    valid = (r_idx <= i_idx)[None]
    D = np.where(valid, x[:, r_idx, cols], np.float32(0.0))  # [B, S(i), S(j)]

    in_maps = []
    for c in range(NCORES):
        Wc = W[c::8]  # [M, S(k), S(j)]
        WIMG = np.empty((128, WTOT), dtype=ndt)
        for m in range(M):
            L, nq = LBAR[m], NQ[m]
            # img[j, (q, k)] = Wc[m, k, 128q + j]
            blk = Wc[m, 0:L, 0 : 128 * nq].astype(ndt, copy=False)  # [k=L, j]
            img = blk.T.reshape(nq, 128, L).transpose(1, 0, 2).reshape(128, nq * L)
            WIMG[:, WCUM[m] : WCUM[m + 1]] = img
        # DT[j, q, m, b] = D[b, 8m+c, 128q+j]
        DT = np.ascontiguousarray(
            D[:, c::8, :].transpose(2, 1, 0).reshape(4, 128, M, B).transpose(1, 0, 2, 3)
        ).astype(ndt, copy=False)
        in_maps.append({"wimg": WIMG, "dt": DT.reshape(128, 4 * M * B)})
    return in_maps


def _postprocess(x, bvec, results):
    """Assemble per-core outputs, add bias, scatter back."""
    out_full = np.empty((B, S, S), dtype=np.float32)
    for c in range(NCORES):
        o = results[c]["out"]  # [128, OTOT]
        for g in range(G):
            blk = o[:, OCUM[g] : OCUM[g + 1]].reshape(4, 32, LG[g])[:, 0:B]
            for t in range(4):
                m = 4 * g + t
                out_full[:, 8 * m + c, 0 : LBAR[m]] = blk[t, :, 0 : LBAR[m]]
    out_full += bvec[None]
    rr = np.arange(S)[:, None]
    cc = np.arange(S)[None, :]
    diag = rr + cc
    new_x = np.where(
        (diag < S)[None], out_full[:, np.minimum(diag, S - 1), cc], x
    ).astype(np.float32)
    return new_x


def kernel_run(x, W, b, mode=MODE, trace=False):
    from concourse.bass_utils import run_bass_kernel_spmd

    nc = _get_program(mode)
    in_maps = _prep_inputs(x, W, mode)
    res = run_bass_kernel_spmd(nc, in_maps, list(range(NCORES)), trace=trace)
    return _postprocess(x, b, res.results), res


def kernel(x, W, b):
    out, _ = kernel_run(np.asarray(x), np.asarray(W), np.asarray(b))
    return out
